# revision 4
# baseline (speedup 1.0000x reference)
"""Hard-negative contrastive loss on 8 TRN2 NeuronCores (Bass/Tile). v5: fp8(e4m3) DoubleRow matmuls, ACT-engine PSUM evacuation for odd row-tiles.

Same math as v2 (bf16 datapath, bisection top-k, host-side logs) with the
AllToAll split into two column-halves so the first exchange overlaps the
second half of the matmul work, and the per-row max/min/exp/mask prep
split per half so it starts as soon as each half of V lands.
"""
import sys

if "/opt/trn_rl_repo" not in sys.path:
    sys.path.insert(0, "/opt/trn_rl_repo")

import numpy as np

N_CORES = 8
B, Q, D = 1024, 32, 512
JQ = (B // N_CORES) * Q        # 4096 target vectors per core
KC = D // 128                  # 4 contraction chunks
NBLK = 512                     # jq per matmul / psum tile
JBLK = NBLK // Q               # 16 j columns per psum tile
NB = JQ // NBLK                # 8 jq blocks
N_ITERS = 10                   # bisection steps
NUM_HARD = B // 2              # 512
NEG_BIG = -1.0e30
HALF = 64                      # j columns per exchange chunk

_RUNNER = None


def _build():
    import concourse.bacc as bacc
    import concourse.mybir as mybir
    import concourse.tile as tile

    f32 = mybir.dt.float32
    bf16 = mybir.dt.bfloat16
    i32 = mybir.dt.int32
    Alu = mybir.AluOpType
    Act = mybir.ActivationFunctionType
    X = mybir.AxisListType.X
    XY = mybir.AxisListType.XY

    nc = bacc.Bacc(None, target_bir_lowering=False, debug=False,
                   num_devices=N_CORES)

    fp8 = mybir.dt.float8e4
    fusT_ap = nc.dram_tensor("fusT", [KC, 128, B], fp8, kind="ExternalInput").ap()
    tgtT_ap = nc.dram_tensor("tgtT", [KC, 128, JQ], fp8, kind="ExternalInput").ap()
    oneh_ap = nc.dram_tensor("onehot", [128, N_CORES, 128], bf16,
                             kind="ExternalInput").ap()
    out_ap = nc.dram_tensor("rowloss", [128, 4], f32, kind="ExternalOutput").ap()

    with tile.TileContext(nc) as tc:
        with (
            tc.tile_pool(name="fus", bufs=1) as fus_pool,
            tc.tile_pool(name="tgt", bufs=2) as tgt_pool,
            tc.tile_pool(name="res", bufs=1) as res_pool,
            tc.tile_pool(name="big", bufs=1) as big_pool,
            tc.tile_pool(name="small", bufs=1) as small_pool,
            tc.tile_pool(name="psum", bufs=8, space="PSUM") as psum_pool,
            tc.tile_pool(name="stg", bufs=3) as stg_pool,
            tc.tile_pool(name="dram", bufs=1, space="DRAM") as dram_pool,
        ):
            # ---------- phase 1: my (1024 x 128) block of v ----------
            fus = fus_pool.tile([128, KC, B], fp8)
            nc.scalar.dma_start(fus[:], fusT_ap.transpose([1, 0, 2]))

            oneh = big_pool.tile([128, N_CORES, 128], bf16)
            nc.scalar.dma_start(oneh[:], oneh_ap[:])

            P_sb = res_pool.tile([128, N_CORES, 128], bf16)  # [i_part, i_tile, j]

            p_in = [dram_pool.tile([N_CORES, 128, HALF], bf16, name=f"p_in{h}")
                    for h in range(2)]
            p_out = [dram_pool.tile([N_CORES, 128, HALF], bf16, name=f"p_out{h}")
                     for h in range(2)]
            V3 = big_pool.tile([128, N_CORES, 128], bf16)

            def exchange(h):
                sl = slice(h * HALF, (h + 1) * HALF)
                nc.sync.dma_start(p_in[h].transpose([1, 0, 2]), P_sb[:, :, sl])
                nc.gpsimd.collective_compute(
                    "AllToAll",
                    Alu.bypass,
                    replica_groups=[list(range(N_CORES))],
                    ins=[p_in[h].opt()],
                    outs=[p_out[h].opt()],
                )
                nc.sync.dma_start(V3[:, :, sl], p_out[h].transpose([1, 0, 2]))

            for b in range(NB):
                tgt = tgt_pool.tile([128, KC, NBLK], fp8)
                nc.sync.dma_start(
                    tgt[:],
                    tgtT_ap[:, :, b * NBLK:(b + 1) * NBLK].transpose([1, 0, 2]))
                for it in range(N_CORES):
                    ps = psum_pool.tile([128, NBLK], f32)
                    for k2 in range(KC // 2):
                        nc.tensor.matmul(
                            ps[:],
                            fus[:, 2 * k2:2 * k2 + 2, it * 128:(it + 1) * 128],
                            tgt[:, 2 * k2:2 * k2 + 2, :],
                            start=(k2 == 0),
                            stop=(k2 == KC // 2 - 1),
                            perf_mode=mybir.MatmulPerfMode.DoubleRow,
                        )
                    if it % 2 == 1 or it == 0:
                        # offload the PSUM read to ACT; DVE reduces from SBUF
                        stg = stg_pool.tile([128, NBLK], bf16)
                        nc.scalar.copy(stg[:], ps[:])
                        red_in = stg.rearrange("p (j q) -> p j q", q=Q)
                    else:
                        red_in = ps.rearrange("p (j q) -> p j q", q=Q)
                    nc.vector.reduce_max(
                        P_sb[:, it, b * JBLK:(b + 1) * JBLK],
                        red_in,
                        axis=X,
                    )
                if b == NB // 2 - 1:
                    exchange(0)   # first half of columns done for all rows
            exchange(1)

            # ---------- phase 2: per-row quantities ----------
            Vmask = big_pool.tile([128, N_CORES, 128], bf16)
            E = big_pool.tile([128, N_CORES, 128], bf16)
            junk = big_pool.tile([128, N_CORES, 128], bf16)

            def sm(name, n=1):
                return small_pool.tile([128, n], f32, name=name, tag=name)

            m2, lo2, pos2, sf2 = (sm(n, 2) for n in "m2 lo2 pos2 sf2".split())
            m, negm, lo, hi, mid, cnt, cnt_hi = (
                sm(n) for n in "m negm lo hi mid cnt cnt_hi".split())
            upd = small_pool.tile([128, 1], i32, name="upd", tag="upd")
            updn = small_pool.tile([128, 1], i32, name="updn", tag="updn")
            pos, sumfull, sumsel, ehi, epos, rem, acc = (
                sm(n) for n in "pos sumfull sumsel ehi epos rem acc".split())

            # per-half prep: max, min, mask, pos-part (overlaps exchange(1))
            for h in range(2):
                sl = slice(h * HALF, (h + 1) * HALF)
                nc.vector.reduce_max(m2[:, h:h + 1], V3[:, :, sl], axis=XY)
                nc.vector.tensor_reduce(lo2[:, h:h + 1], V3[:, :, sl], axis=XY,
                                        op=Alu.min)
                nc.vector.scalar_tensor_tensor(
                    Vmask[:, :, sl], oneh[:, :, sl], NEG_BIG, V3[:, :, sl],
                    op0=Alu.mult, op1=Alu.add)
                nc.vector.scalar_tensor_tensor(
                    junk[:, :, sl], oneh[:, :, sl], 1.0, V3[:, :, sl],
                    op0=Alu.mult, op1=Alu.mult, accum_out=pos2[:, h:h + 1])

            nc.vector.reduce_max(m[:], m2[:], axis=X)
            nc.vector.tensor_scalar_mul(negm[:], m[:], -1.0)
            nc.vector.tensor_reduce(lo[:], lo2[:], axis=X, op=Alu.min)
            nc.vector.tensor_scalar_add(lo[:], lo[:], -1.0)
            nc.vector.tensor_copy(hi[:], m[:])
            nc.vector.memset(cnt_hi[:], 0.0)
            nc.vector.tensor_add(pos[:], pos2[:, 0:1], pos2[:, 1:2])

            # E = exp(V - m), sumfull = sum_j E (needs final m -> after both halves)
            nc.scalar.activation(E[:], V3[:], Act.Exp, bias=negm[:], scale=1.0,
                                 accum_out=sumfull[:])

            Vflat = Vmask.rearrange("p s j -> p (s j)")
            Eflat = E.rearrange("p s j -> p (s j)")
            jflat = junk.rearrange("p s j -> p (s j)")

            # bisection for the top-512 threshold
            for _ in range(N_ITERS):
                nc.vector.tensor_add(mid[:], lo[:], hi[:])
                nc.vector.tensor_scalar_mul(mid[:], mid[:], 0.5)
                nc.vector.tensor_scalar(
                    jflat, Vflat, mid[:], None, op0=Alu.is_gt,
                    op1=Alu.add, accum_out=cnt[:])
                nc.vector.tensor_scalar(upd[:], cnt[:], float(NUM_HARD), None,
                                        op0=Alu.is_gt)
                nc.vector.tensor_scalar(updn[:], cnt[:], float(NUM_HARD), None,
                                        op0=Alu.is_le)
                nc.vector.copy_predicated(lo[:], upd[:], mid[:])
                nc.vector.copy_predicated(hi[:], updn[:], mid[:])
                nc.vector.copy_predicated(cnt_hi[:], updn[:], cnt[:])

            # sumsel = sum E over entries with v > hi (cnt_hi of them);
            # remaining (512 - cnt_hi) entries lie in (lo, hi] ~ exp(hi) each
            nc.vector.scalar_tensor_tensor(
                jflat, Vflat, hi[:], Eflat, op0=Alu.is_gt, op1=Alu.mult,
                accum_out=sumsel[:])

            nc.scalar.activation(epos[:], pos[:], Act.Exp, bias=negm[:])
            nc.scalar.activation(ehi[:], hi[:], Act.Exp, bias=negm[:])
            # rem = 512 - cnt_hi ; acc = epos + sumsel + rem * ehi
            nc.vector.tensor_scalar(rem[:], cnt_hi[:], -1.0, float(NUM_HARD),
                                    op0=Alu.mult, op1=Alu.add)
            nc.vector.tensor_mul(rem[:], rem[:], ehi[:])
            nc.vector.tensor_add(acc[:], epos[:], sumsel[:])
            nc.vector.tensor_add(acc[:], acc[:], rem[:])

            # outputs: m, pos, sumfull, acc (host takes logs and averages)
            outs = res_pool.tile([128, 4], f32)
            nc.vector.tensor_copy(outs[:, 0:1], m[:])
            nc.vector.tensor_copy(outs[:, 1:2], pos[:])
            nc.vector.tensor_copy(outs[:, 2:3], sumfull[:])
            nc.vector.tensor_copy(outs[:, 3:4], acc[:])

            nc.sync.dma_start(out_ap[:], outs[:])

    nc.compile()
    return nc


def _get_nc():
    global _RUNNER
    if _RUNNER is None:
        _RUNNER = _build()
    return _RUNNER


def make_in_maps(fusion_feats, target_feats, temp):
    import ml_dtypes

    bf16 = ml_dtypes.bfloat16
    fp8 = ml_dtypes.float8_e4m3
    fusion = np.asarray(fusion_feats, dtype=np.float32)
    target = np.asarray(target_feats, dtype=np.float32)
    scale = np.float32(1.0 / float(np.asarray(temp)))
    fusT = np.ascontiguousarray((fusion * scale).T).reshape(KC, 128, B)
    fusT = fusT.astype(fp8)
    rows_per = B // N_CORES
    in_maps = []
    for c in range(N_CORES):
        shard = target[c * rows_per:(c + 1) * rows_per].reshape(JQ, D)
        tgtT = np.ascontiguousarray(shard.T).reshape(KC, 128, JQ).astype(fp8)
        onehot = np.zeros((rows_per, B), dtype=np.float32)
        onehot[np.arange(rows_per), c * rows_per + np.arange(rows_per)] = 1.0
        in_maps.append({"fusT": fusT, "tgtT": tgtT,
                        "onehot": onehot.astype(bf16).reshape(128, N_CORES, 128)})
    return in_maps


def combine(results):
    rows = np.concatenate(
        [np.asarray(r["rowloss"], dtype=np.float32) for r in results], axis=0)
    m, pos, sumfull, acc = rows[:, 0], rows[:, 1], rows[:, 2], rows[:, 3]
    loss_std = (m + np.log(sumfull) - pos).mean(dtype=np.float32)
    loss_hard = (m + np.log(acc) - pos).mean(dtype=np.float32)
    loss = loss_std + np.float32(0.5) * loss_hard
    return np.asarray(loss, dtype=np.float32)


def kernel(fusion_feats, target_feats, temp):
    from concourse import bass_utils

    nc = _get_nc()
    in_maps = make_in_maps(fusion_feats, target_feats, temp)
    res = bass_utils.run_bass_kernel_spmd(nc, in_maps, list(range(N_CORES)))
    return combine(res.results)


# revision 5
# speedup vs baseline: 1.0420x; 1.0420x over previous
"""Hard-negative contrastive loss on 8 TRN2 NeuronCores (Bass/Tile). v5: fp8(e4m3) DoubleRow matmuls, ACT-engine PSUM evacuation for odd row-tiles.

Same math as v2 (bf16 datapath, bisection top-k, host-side logs) with the
AllToAll split into two column-halves so the first exchange overlaps the
second half of the matmul work, and the per-row max/min/exp/mask prep
split per half so it starts as soon as each half of V lands.
"""
import sys

if "/opt/trn_rl_repo" not in sys.path:
    sys.path.insert(0, "/opt/trn_rl_repo")

import numpy as np

N_CORES = 8
B, Q, D = 1024, 32, 512
JQ = (B // N_CORES) * Q        # 4096 target vectors per core
KC = D // 128                  # 4 contraction chunks
NBLK = 512                     # jq per matmul / psum tile
JBLK = NBLK // Q               # 16 j columns per psum tile
NB = JQ // NBLK                # 8 jq blocks
N_ITERS = 10                   # bisection steps
NUM_HARD = B // 2              # 512
NEG_BIG = -1.0e30
HALF = 64                      # j columns per exchange chunk

_RUNNER = None


def _build():
    import concourse.bacc as bacc
    import concourse.mybir as mybir
    import concourse.tile as tile

    f32 = mybir.dt.float32
    bf16 = mybir.dt.bfloat16
    i32 = mybir.dt.int32
    Alu = mybir.AluOpType
    Act = mybir.ActivationFunctionType
    X = mybir.AxisListType.X
    XY = mybir.AxisListType.XY

    nc = bacc.Bacc(None, target_bir_lowering=False, debug=False,
                   num_devices=N_CORES)

    fp8 = mybir.dt.float8e4
    fusT_ap = nc.dram_tensor("fusT", [KC, 128, B], fp8, kind="ExternalInput").ap()
    tgtT_ap = nc.dram_tensor("tgtT", [KC, 128, JQ], fp8, kind="ExternalInput").ap()
    oneh_ap = nc.dram_tensor("onehot", [128, N_CORES, 128], bf16,
                             kind="ExternalInput").ap()
    out_ap = nc.dram_tensor("rowloss", [128, 4], f32, kind="ExternalOutput").ap()

    with tile.TileContext(nc) as tc:
        with (
            tc.tile_pool(name="fus", bufs=1) as fus_pool,
            tc.tile_pool(name="tgt", bufs=2) as tgt_pool,
            tc.tile_pool(name="res", bufs=1) as res_pool,
            tc.tile_pool(name="big", bufs=1) as big_pool,
            tc.tile_pool(name="small", bufs=1) as small_pool,
            tc.tile_pool(name="psum", bufs=8, space="PSUM") as psum_pool,
            tc.tile_pool(name="stg", bufs=3) as stg_pool,
            tc.tile_pool(name="dram", bufs=1, space="DRAM") as dram_pool,
        ):
            # ---------- phase 1: my (1024 x 128) block of v ----------
            fus = fus_pool.tile([128, KC, B], fp8)
            nc.scalar.dma_start(fus[:], fusT_ap.transpose([1, 0, 2]))

            oneh = big_pool.tile([128, N_CORES, 128], bf16)
            nc.scalar.dma_start(oneh[:], oneh_ap[:])

            P_sb = res_pool.tile([128, N_CORES, 128], bf16)  # [i_part, i_tile, j]

            p_in = [dram_pool.tile([N_CORES, 128, HALF], bf16, name=f"p_in{h}")
                    for h in range(2)]
            p_out = [dram_pool.tile([N_CORES, 128, HALF], bf16, name=f"p_out{h}")
                     for h in range(2)]
            V3 = big_pool.tile([128, N_CORES, 128], bf16)

            def exchange(h):
                sl = slice(h * HALF, (h + 1) * HALF)
                nc.sync.dma_start(p_in[h].transpose([1, 0, 2]), P_sb[:, :, sl])
                nc.gpsimd.collective_compute(
                    "AllToAll",
                    Alu.bypass,
                    replica_groups=[list(range(N_CORES))],
                    ins=[p_in[h].opt()],
                    outs=[p_out[h].opt()],
                )
                nc.sync.dma_start(V3[:, :, sl], p_out[h].transpose([1, 0, 2]))

            for b in range(NB):
                tgt = tgt_pool.tile([128, KC, NBLK], fp8)
                nc.sync.dma_start(
                    tgt[:],
                    tgtT_ap[:, :, b * NBLK:(b + 1) * NBLK].transpose([1, 0, 2]))
                for it in range(N_CORES):
                    ps = psum_pool.tile([128, NBLK], f32)
                    for k2 in range(KC // 2):
                        nc.tensor.matmul(
                            ps[:],
                            fus[:, 2 * k2:2 * k2 + 2, it * 128:(it + 1) * 128],
                            tgt[:, 2 * k2:2 * k2 + 2, :],
                            start=(k2 == 0),
                            stop=(k2 == KC // 2 - 1),
                            perf_mode=mybir.MatmulPerfMode.DoubleRow,
                        )
                    if it % 2 == 1:
                        # offload the PSUM read to ACT; DVE reduces from SBUF
                        stg = stg_pool.tile([128, NBLK], bf16)
                        nc.scalar.copy(stg[:], ps[:])
                        red_in = stg.rearrange("p (j q) -> p j q", q=Q)
                    else:
                        red_in = ps.rearrange("p (j q) -> p j q", q=Q)
                    nc.vector.reduce_max(
                        P_sb[:, it, b * JBLK:(b + 1) * JBLK],
                        red_in,
                        axis=X,
                    )
                if b == NB // 2 - 1:
                    exchange(0)   # first half of columns done for all rows
            exchange(1)

            # ---------- phase 2: per-row quantities ----------
            Vmask = big_pool.tile([128, N_CORES, 128], bf16)
            E = big_pool.tile([128, N_CORES, 128], bf16)
            junk = big_pool.tile([128, N_CORES, 128], bf16)

            def sm(name, n=1):
                return small_pool.tile([128, n], f32, name=name, tag=name)

            m2, lo2, pos2, sf2 = (sm(n, 2) for n in "m2 lo2 pos2 sf2".split())
            m, negm, lo, hi, mid, cnt, cnt_hi = (
                sm(n) for n in "m negm lo hi mid cnt cnt_hi".split())
            upd = small_pool.tile([128, 1], i32, name="upd", tag="upd")
            updn = small_pool.tile([128, 1], i32, name="updn", tag="updn")
            pos, sumfull, sumsel, ehi, epos, rem, acc = (
                sm(n) for n in "pos sumfull sumsel ehi epos rem acc".split())

            # per-half prep: max, min, mask, pos-part (overlaps exchange(1))
            for h in range(2):
                sl = slice(h * HALF, (h + 1) * HALF)
                nc.vector.reduce_max(m2[:, h:h + 1], V3[:, :, sl], axis=XY)
                nc.vector.tensor_reduce(lo2[:, h:h + 1], V3[:, :, sl], axis=XY,
                                        op=Alu.min)
                nc.vector.scalar_tensor_tensor(
                    Vmask[:, :, sl], oneh[:, :, sl], NEG_BIG, V3[:, :, sl],
                    op0=Alu.mult, op1=Alu.add)
                nc.vector.scalar_tensor_tensor(
                    junk[:, :, sl], oneh[:, :, sl], 1.0, V3[:, :, sl],
                    op0=Alu.mult, op1=Alu.mult, accum_out=pos2[:, h:h + 1])

            nc.vector.reduce_max(m[:], m2[:], axis=X)
            nc.vector.tensor_scalar_mul(negm[:], m[:], -1.0)
            nc.vector.tensor_reduce(lo[:], lo2[:], axis=X, op=Alu.min)
            nc.vector.tensor_scalar_add(lo[:], lo[:], -1.0)
            nc.vector.tensor_copy(hi[:], m[:])
            nc.vector.memset(cnt_hi[:], 0.0)
            nc.vector.tensor_add(pos[:], pos2[:, 0:1], pos2[:, 1:2])

            # E = exp(V - m), sumfull = sum_j E (needs final m -> after both halves)
            nc.scalar.activation(E[:], V3[:], Act.Exp, bias=negm[:], scale=1.0,
                                 accum_out=sumfull[:])

            Vflat = Vmask.rearrange("p s j -> p (s j)")
            Eflat = E.rearrange("p s j -> p (s j)")
            jflat = junk.rearrange("p s j -> p (s j)")

            # bisection for the top-512 threshold
            for _ in range(N_ITERS):
                nc.vector.tensor_add(mid[:], lo[:], hi[:])
                nc.vector.tensor_scalar_mul(mid[:], mid[:], 0.5)
                nc.vector.tensor_scalar(
                    jflat, Vflat, mid[:], None, op0=Alu.is_gt,
                    op1=Alu.add, accum_out=cnt[:])
                nc.vector.tensor_scalar(upd[:], cnt[:], float(NUM_HARD), None,
                                        op0=Alu.is_gt)
                nc.vector.tensor_scalar(updn[:], cnt[:], float(NUM_HARD), None,
                                        op0=Alu.is_le)
                nc.vector.copy_predicated(lo[:], upd[:], mid[:])
                nc.vector.copy_predicated(hi[:], updn[:], mid[:])
                nc.vector.copy_predicated(cnt_hi[:], updn[:], cnt[:])

            # sumsel = sum E over entries with v > hi (cnt_hi of them);
            # remaining (512 - cnt_hi) entries lie in (lo, hi] ~ exp(hi) each
            nc.vector.scalar_tensor_tensor(
                jflat, Vflat, hi[:], Eflat, op0=Alu.is_gt, op1=Alu.mult,
                accum_out=sumsel[:])

            nc.scalar.activation(epos[:], pos[:], Act.Exp, bias=negm[:])
            nc.scalar.activation(ehi[:], hi[:], Act.Exp, bias=negm[:])
            # rem = 512 - cnt_hi ; acc = epos + sumsel + rem * ehi
            nc.vector.tensor_scalar(rem[:], cnt_hi[:], -1.0, float(NUM_HARD),
                                    op0=Alu.mult, op1=Alu.add)
            nc.vector.tensor_mul(rem[:], rem[:], ehi[:])
            nc.vector.tensor_add(acc[:], epos[:], sumsel[:])
            nc.vector.tensor_add(acc[:], acc[:], rem[:])

            # outputs: m, pos, sumfull, acc (host takes logs and averages)
            outs = res_pool.tile([128, 4], f32)
            nc.vector.tensor_copy(outs[:, 0:1], m[:])
            nc.vector.tensor_copy(outs[:, 1:2], pos[:])
            nc.vector.tensor_copy(outs[:, 2:3], sumfull[:])
            nc.vector.tensor_copy(outs[:, 3:4], acc[:])

            nc.sync.dma_start(out_ap[:], outs[:])

    nc.compile()
    return nc


def _get_nc():
    global _RUNNER
    if _RUNNER is None:
        _RUNNER = _build()
    return _RUNNER


def make_in_maps(fusion_feats, target_feats, temp):
    import ml_dtypes

    bf16 = ml_dtypes.bfloat16
    fp8 = ml_dtypes.float8_e4m3
    fusion = np.asarray(fusion_feats, dtype=np.float32)
    target = np.asarray(target_feats, dtype=np.float32)
    scale = np.float32(1.0 / float(np.asarray(temp)))
    fusT = np.ascontiguousarray((fusion * scale).T).reshape(KC, 128, B)
    fusT = fusT.astype(fp8)
    rows_per = B // N_CORES
    in_maps = []
    for c in range(N_CORES):
        shard = target[c * rows_per:(c + 1) * rows_per].reshape(JQ, D)
        tgtT = np.ascontiguousarray(shard.T).reshape(KC, 128, JQ).astype(fp8)
        onehot = np.zeros((rows_per, B), dtype=np.float32)
        onehot[np.arange(rows_per), c * rows_per + np.arange(rows_per)] = 1.0
        in_maps.append({"fusT": fusT, "tgtT": tgtT,
                        "onehot": onehot.astype(bf16).reshape(128, N_CORES, 128)})
    return in_maps


def combine(results):
    rows = np.concatenate(
        [np.asarray(r["rowloss"], dtype=np.float32) for r in results], axis=0)
    m, pos, sumfull, acc = rows[:, 0], rows[:, 1], rows[:, 2], rows[:, 3]
    loss_std = (m + np.log(sumfull) - pos).mean(dtype=np.float32)
    loss_hard = (m + np.log(acc) - pos).mean(dtype=np.float32)
    loss = loss_std + np.float32(0.5) * loss_hard
    return np.asarray(loss, dtype=np.float32)


def kernel(fusion_feats, target_feats, temp):
    from concourse import bass_utils

    nc = _get_nc()
    in_maps = make_in_maps(fusion_feats, target_feats, temp)
    res = bass_utils.run_bass_kernel_spmd(nc, in_maps, list(range(N_CORES)))
    return combine(res.results)


# revision 6
# speedup vs baseline: 1.0852x; 1.0414x over previous
"""Hard-negative contrastive loss on 8 TRN2 NeuronCores (Bass/Tile). v6: paired-bank PSUM evacuation, 8 bisection steps, in-place output columns.

Same math as v2 (bf16 datapath, bisection top-k, host-side logs) with the
AllToAll split into two column-halves so the first exchange overlaps the
second half of the matmul work, and the per-row max/min/exp/mask prep
split per half so it starts as soon as each half of V lands.
"""
import sys

if "/opt/trn_rl_repo" not in sys.path:
    sys.path.insert(0, "/opt/trn_rl_repo")

import numpy as np

N_CORES = 8
B, Q, D = 1024, 32, 512
JQ = (B // N_CORES) * Q        # 4096 target vectors per core
KC = D // 128                  # 4 contraction chunks
NBLK = 512                     # jq per matmul / psum tile
JBLK = NBLK // Q               # 16 j columns per psum tile
NB = JQ // NBLK                # 8 jq blocks
N_ITERS = 8                   # bisection steps
NUM_HARD = B // 2              # 512
NEG_BIG = -1.0e30
HALF = 64                      # j columns per exchange chunk

_RUNNER = None


def _build():
    import concourse.bacc as bacc
    import concourse.mybir as mybir
    import concourse.tile as tile

    f32 = mybir.dt.float32
    bf16 = mybir.dt.bfloat16
    i32 = mybir.dt.int32
    Alu = mybir.AluOpType
    Act = mybir.ActivationFunctionType
    X = mybir.AxisListType.X
    XY = mybir.AxisListType.XY

    nc = bacc.Bacc(None, target_bir_lowering=False, debug=False,
                   num_devices=N_CORES)

    fp8 = mybir.dt.float8e4
    fusT_ap = nc.dram_tensor("fusT", [KC, 128, B], fp8, kind="ExternalInput").ap()
    tgtT_ap = nc.dram_tensor("tgtT", [KC, 128, JQ], fp8, kind="ExternalInput").ap()
    oneh_ap = nc.dram_tensor("onehot", [128, N_CORES, 128], bf16,
                             kind="ExternalInput").ap()
    out_ap = nc.dram_tensor("rowloss", [128, 4], f32, kind="ExternalOutput").ap()

    with tile.TileContext(nc) as tc:
        with (
            tc.tile_pool(name="fus", bufs=1) as fus_pool,
            tc.tile_pool(name="tgt", bufs=2) as tgt_pool,
            tc.tile_pool(name="res", bufs=1) as res_pool,
            tc.tile_pool(name="big", bufs=1) as big_pool,
            tc.tile_pool(name="small", bufs=1) as small_pool,
            tc.tile_pool(name="psum", bufs=4, space="PSUM") as psum_pool,
            tc.tile_pool(name="stg", bufs=3) as stg_pool,
            tc.tile_pool(name="dram", bufs=1, space="DRAM") as dram_pool,
        ):
            # ---------- phase 1: my (1024 x 128) block of v ----------
            fus = fus_pool.tile([128, KC, B], fp8)
            nc.scalar.dma_start(fus[:], fusT_ap.transpose([1, 0, 2]))

            oneh = big_pool.tile([128, N_CORES, 128], bf16)
            nc.scalar.dma_start(oneh[:], oneh_ap[:])

            P_sb = res_pool.tile([128, N_CORES, 128], bf16)  # [i_part, i_tile, j]

            p_in = [dram_pool.tile([N_CORES, 128, HALF], bf16, name=f"p_in{h}")
                    for h in range(2)]
            p_out = [dram_pool.tile([N_CORES, 128, HALF], bf16, name=f"p_out{h}")
                     for h in range(2)]
            V3 = big_pool.tile([128, N_CORES, 128], bf16)

            def exchange(h):
                sl = slice(h * HALF, (h + 1) * HALF)
                nc.sync.dma_start(p_in[h].transpose([1, 0, 2]), P_sb[:, :, sl])
                nc.gpsimd.collective_compute(
                    "AllToAll",
                    Alu.bypass,
                    replica_groups=[list(range(N_CORES))],
                    ins=[p_in[h].opt()],
                    outs=[p_out[h].opt()],
                )
                nc.sync.dma_start(V3[:, :, sl], p_out[h].transpose([1, 0, 2]))

            for b in range(NB):
                tgt = tgt_pool.tile([128, KC, NBLK], fp8)
                nc.sync.dma_start(
                    tgt[:],
                    tgtT_ap[:, :, b * NBLK:(b + 1) * NBLK].transpose([1, 0, 2]))
                for t in range(N_CORES // 2):
                    # two row-tiles share one 2-bank psum tile so each
                    # evacuation op (ACT copy / DVE reduce) covers both
                    ps = psum_pool.tile([128, 2, NBLK], f32)
                    for half in range(2):
                        it = 2 * t + half
                        for k2 in range(KC // 2):
                            nc.tensor.matmul(
                                ps[:, half, :],
                                fus[:, 2 * k2:2 * k2 + 2, it * 128:(it + 1) * 128],
                                tgt[:, 2 * k2:2 * k2 + 2, :],
                                start=(k2 == 0),
                                stop=(k2 == KC // 2 - 1),
                                perf_mode=mybir.MatmulPerfMode.DoubleRow,
                            )
                    if t % 2 == 1:
                        # offload the PSUM read to ACT; DVE reduces from SBUF
                        stg = stg_pool.tile([128, 2, NBLK], bf16)
                        nc.scalar.copy(stg[:], ps[:])
                        red_in = stg.rearrange("p i (j q) -> p i j q", q=Q)
                    else:
                        red_in = ps.rearrange("p i (j q) -> p i j q", q=Q)
                    nc.vector.reduce_max(
                        P_sb[:, 2 * t:2 * t + 2, b * JBLK:(b + 1) * JBLK],
                        red_in,
                        axis=X,
                    )
                if b == NB // 2 - 1:
                    exchange(0)   # first half of columns done for all rows
            exchange(1)

            # ---------- phase 2: per-row quantities ----------
            Vmask = big_pool.tile([128, N_CORES, 128], bf16)
            E = big_pool.tile([128, N_CORES, 128], bf16)
            junk = big_pool.tile([128, N_CORES, 128], bf16)

            def sm(name, n=1):
                return small_pool.tile([128, n], f32, name=name, tag=name)

            m2, lo2, pos2 = (sm(n, 2) for n in "m2 lo2 pos2".split())
            negm, lo, hi, mid, cnt, cnt_hi = (
                sm(n) for n in "negm lo hi mid cnt cnt_hi".split())
            upd = small_pool.tile([128, 1], i32, name="upd", tag="upd")
            updn = small_pool.tile([128, 1], i32, name="updn", tag="updn")
            sumsel, ehi, epos, rem = (
                sm(n) for n in "sumsel ehi epos rem".split())
            outs = res_pool.tile([128, 4], f32)
            m, pos, sumfull, acc = (outs[:, c:c + 1] for c in range(4))

            # per-half prep: max, min, mask, pos-part (overlaps exchange(1))
            for h in range(2):
                sl = slice(h * HALF, (h + 1) * HALF)
                nc.vector.reduce_max(m2[:, h:h + 1], V3[:, :, sl], axis=XY)
                nc.vector.tensor_reduce(lo2[:, h:h + 1], V3[:, :, sl], axis=XY,
                                        op=Alu.min)
                nc.vector.scalar_tensor_tensor(
                    Vmask[:, :, sl], oneh[:, :, sl], NEG_BIG, V3[:, :, sl],
                    op0=Alu.mult, op1=Alu.add)
                nc.vector.scalar_tensor_tensor(
                    junk[:, :, sl], oneh[:, :, sl], 1.0, V3[:, :, sl],
                    op0=Alu.mult, op1=Alu.mult, accum_out=pos2[:, h:h + 1])

            nc.vector.reduce_max(m, m2[:], axis=X)
            nc.vector.tensor_scalar_mul(negm[:], m, -1.0)
            nc.vector.tensor_reduce(lo[:], lo2[:], axis=X, op=Alu.min)
            nc.vector.tensor_scalar_add(lo[:], lo[:], -1.0)
            nc.vector.tensor_copy(hi[:], m)
            nc.vector.memset(cnt_hi[:], 0.0)
            nc.vector.tensor_add(pos, pos2[:, 0:1], pos2[:, 1:2])

            # E = exp(V - m), sumfull = sum_j E (needs final m -> after both halves)
            nc.scalar.activation(E[:], V3[:], Act.Exp, bias=negm[:], scale=1.0,
                                 accum_out=sumfull)

            Vflat = Vmask.rearrange("p s j -> p (s j)")
            Eflat = E.rearrange("p s j -> p (s j)")
            jflat = junk.rearrange("p s j -> p (s j)")

            # bisection for the top-512 threshold
            for _ in range(N_ITERS):
                nc.vector.tensor_add(mid[:], lo[:], hi[:])
                nc.vector.tensor_scalar_mul(mid[:], mid[:], 0.5)
                nc.vector.tensor_scalar(
                    jflat, Vflat, mid[:], None, op0=Alu.is_gt,
                    op1=Alu.add, accum_out=cnt[:])
                nc.vector.tensor_scalar(upd[:], cnt[:], float(NUM_HARD), None,
                                        op0=Alu.is_gt)
                nc.vector.tensor_scalar(updn[:], cnt[:], float(NUM_HARD), None,
                                        op0=Alu.is_le)
                nc.vector.copy_predicated(lo[:], upd[:], mid[:])
                nc.vector.copy_predicated(hi[:], updn[:], mid[:])
                nc.vector.copy_predicated(cnt_hi[:], updn[:], cnt[:])

            # sumsel = sum E over entries with v > hi (cnt_hi of them);
            # remaining (512 - cnt_hi) entries lie in (lo, hi] ~ exp(hi) each
            nc.vector.scalar_tensor_tensor(
                jflat, Vflat, hi[:], Eflat, op0=Alu.is_gt, op1=Alu.mult,
                accum_out=sumsel[:])

            nc.scalar.activation(epos[:], pos, Act.Exp, bias=negm[:])
            nc.scalar.activation(ehi[:], hi[:], Act.Exp, bias=negm[:])
            # rem = 512 - cnt_hi ; acc = epos + sumsel + rem * ehi
            nc.vector.tensor_scalar(rem[:], cnt_hi[:], -1.0, float(NUM_HARD),
                                    op0=Alu.mult, op1=Alu.add)
            nc.vector.tensor_mul(rem[:], rem[:], ehi[:])
            nc.vector.tensor_add(acc, epos[:], sumsel[:])
            nc.vector.tensor_add(acc, acc, rem[:])

            nc.sync.dma_start(out_ap[:], outs[:])

    nc.compile()
    return nc


def _get_nc():
    global _RUNNER
    if _RUNNER is None:
        _RUNNER = _build()
    return _RUNNER


def make_in_maps(fusion_feats, target_feats, temp):
    import ml_dtypes

    bf16 = ml_dtypes.bfloat16
    fp8 = ml_dtypes.float8_e4m3
    fusion = np.asarray(fusion_feats, dtype=np.float32)
    target = np.asarray(target_feats, dtype=np.float32)
    scale = np.float32(1.0 / float(np.asarray(temp)))
    fusT = np.ascontiguousarray((fusion * scale).T).reshape(KC, 128, B)
    fusT = fusT.astype(fp8)
    rows_per = B // N_CORES
    in_maps = []
    for c in range(N_CORES):
        shard = target[c * rows_per:(c + 1) * rows_per].reshape(JQ, D)
        tgtT = np.ascontiguousarray(shard.T).reshape(KC, 128, JQ).astype(fp8)
        onehot = np.zeros((rows_per, B), dtype=np.float32)
        onehot[np.arange(rows_per), c * rows_per + np.arange(rows_per)] = 1.0
        in_maps.append({"fusT": fusT, "tgtT": tgtT,
                        "onehot": onehot.astype(bf16).reshape(128, N_CORES, 128)})
    return in_maps


def combine(results):
    rows = np.concatenate(
        [np.asarray(r["rowloss"], dtype=np.float32) for r in results], axis=0)
    m, pos, sumfull, acc = rows[:, 0], rows[:, 1], rows[:, 2], rows[:, 3]
    loss_std = (m + np.log(sumfull) - pos).mean(dtype=np.float32)
    loss_hard = (m + np.log(acc) - pos).mean(dtype=np.float32)
    loss = loss_std + np.float32(0.5) * loss_hard
    return np.asarray(loss, dtype=np.float32)


def kernel(fusion_feats, target_feats, temp):
    from concourse import bass_utils

    nc = _get_nc()
    in_maps = make_in_maps(fusion_feats, target_feats, temp)
    res = bass_utils.run_bass_kernel_spmd(nc, in_maps, list(range(N_CORES)))
    return combine(res.results)
